# revision 35
# baseline (speedup 1.0000x reference)
"""Multi-head self-attention (causal) Trainium2 Bass kernel, 8-core SPMD.

Problem: B=2, S=2048, D_IN=1024, D_OUT=1024, H=16 heads (hd=64), causal
softmax, out-proj with bias.

Sharding: core c -> (batch b = c // 4, head-group g = c % 4). Each core
computes 4 heads of one batch: data-parallel over b, tensor-parallel over
heads (W_q/W_k/W_v column shards, W_o row shard). Host sums the 4 partial
out-proj results per batch and adds b_o.

On-core layout is fully transposed (feature-major):
  xT   [D_IN, S]                (host pre-transposes x[b])
  Qt,Kt[hd*4, S]  = W^T @ xT    (pair-major: 2 sbuf tiles of [128, S])
  V    [S, hd*4]  (normal orientation, 64 ones-columns PREPENDED per
                   head: the ctx matmul emits the softmax denominator
                   replicated across PSUM partitions 0-63, so the
                   reciprocal reads PSUM directly at partition offset 0)
  St   [k, q] scores transposed; Pt = exp(St/8) in bf16, causal-masked
  in-place by a Pool-engine affine_select (fill=0) on diagonal blocks
  ctxT [hd*4, S] normalized via reciprocal_approx_fast straight from
  PSUM (no staging copies)
  outT [D_OUT, S] partial = Wo_shard^T @ ctxT  (host transposes back)

All matmul operands bf16 (fp32 PSUM accumulate); softmax in fp32.
Scores for a head pair are packed: head0 on PE row-groups 0-1, head1 on
2-3 (concurrent matmuls).

Startup is DMA-software-pipelined over THREE issue queues (sync/
scalar/gpsimd): wq/wk pair-0 stream finest-first, xT seq chunk 0 as
per-ki 128KB pieces, a junk-matmul burst engages the HAM clock ramp
with no idle gap (an idle >~0.5us resets the ramp), and the Q0/K0
projection chains chase the incoming pieces so real attention starts
~12us in (vs waiting for the full 6MB input). The two head-pairs' seq
chunks are interleaved into ONE global pipeline (chunk order
p0j0,p0j1,p1j0,p0j2,p1j1,p1j2,p0j3,p1j3) so the small latency-bound
chunks hide inside the other pair's dense chunks and the scalar-engine
exp load stays smooth. Scores pairs are emitted in bursts of two
(fillers + ctx retires only after odd units): entering/exiting a
row-group-packed pair costs ~100ns of non-hidden LDWEIGHTS each, so
bursting halves that penalty; burst depth is capped at 2 by the two
scores PSUM tiles, whose recycling waits on the serial scalar exp
chain. ctx matmuls retire with lag behind exp (pt ring of 8) so
select/exp latency never blocks the in-order PE stream. QKV
projection / out-projection tiles are interleaved as deadline-ordered
fillers with positional forced flushes; four held-back out-proj tiles
bridge the PE over the final softmax normalize, and tail copies/DMA
issues fan out over scalar+vector engines and all three DMA queues.
"""

import numpy as np
import ml_dtypes

import concourse.bass as bass
import concourse.bacc as bacc
import concourse.tile as tile
import concourse.mybir as mybir
from concourse.bass_utils import run_bass_kernel_spmd

N_CORES = 8
B, S, D_IN, D_OUT, H = 2, 2048, 1024, 1024, 16
H_LOC = 4  # heads per core
HD = 64
DH = H_LOC * HD  # 256 = d_out shard per core
KI = D_IN // 128  # 8 contraction chunks
NQ = S // 512  # 4 seq chunks of 512
NB = S // 128  # 16 seq blocks of 128
SCALE = 1.0 / np.sqrt(np.float32(HD))  # 0.125

BF16 = mybir.dt.bfloat16
F32 = mybir.dt.float32
EXP = mybir.ActivationFunctionType.Exp


def _pair_view(ap2d):
    """[128, 1024] tile -> [128, 2, 512] (head-major) view."""
    return ap2d.rearrange("p (h q) -> p h q", h=2)


def _build_body(nc, tc, xT_d, wq_d, wk_d, wv_d, wo_d, outT_d):
    from contextlib import ExitStack

    ctx = ExitStack()
    const = ctx.enter_context(tc.tile_pool(name="const", bufs=1))
    # PSUM: sc [128,1024]x2 = 4 banks; ctx [128,1024]x1 = 2; qk [128,512]x2 = 2
    sc_ps = ctx.enter_context(tc.tile_pool(name="sc_ps", bufs=2, space="PSUM"))
    ctx_ps = ctx.enter_context(tc.tile_pool(name="ctx_ps", bufs=1, space="PSUM"))
    qk_ps = ctx.enter_context(tc.tile_pool(name="qk_ps", bufs=2, space="PSUM"))
    pt_pool = ctx.enter_context(tc.tile_pool(name="pt", bufs=8))
    ob_pool = ctx.enter_context(tc.tile_pool(name="ob", bufs=6))
    r64_pool = ctx.enter_context(tc.tile_pool(name="r64", bufs=2))

    # ---- resident SBUF tensors ----
    xT_s = const.tile([128, NQ, KI, 512], BF16)
    wq_s = const.tile([128, 2, KI, 128], BF16)
    wk_s = const.tile([128, 2, KI, 128], BF16)
    wv_s = const.tile([128, KI, DH], BF16)
    wo_s = const.tile([128, 2, D_OUT], BF16)
    qt_s = const.tile([128, 2, S], BF16)  # [64*sub + hd, pair, q]
    kt_s = const.tile([128, 2, S], BF16)
    v_s = const.tile([128, NB, H_LOC, 2 * HD], BF16)  # cols 0:HD = ones x64
    ctxT_s = const.tile([128, 2, S], BF16)
    wu = const.tile([128, 512], BF16)

    # ---- input DMAs: seq-major pieces so Q0/K0 can start early.
    # FOUR issue queues (sync/scalar/vector/gpsimd). Seq chunk 0 streams
    # as per-ki 128KB pieces split across the vector+gpsimd queues so the
    # Q0/K0 contraction chains chase arrival; weights pair-0 pieces go
    # finest-first on sync/scalar; wv right after so the first V blocks
    # unblock ctx of chunk 0 quickly; wo goes last. ----
    xv = xT_d.rearrange("p (s k c) -> p s k c", s=NQ, k=KI)
    wqv = wq_d.rearrange("p (r k c) -> p r k c", r=2, k=KI)
    wkv = wk_d.rearrange("p (r k c) -> p r k c", r=2, k=KI)
    wvv = wv_d.rearrange("p (c s) -> p c s", c=KI)
    # junk-matmul operand must be set before gpsimd's DMA issues so the
    # PE ramp starts immediately
    nc.gpsimd.memset(wu, 0.0)
    nc.sync.dma_start(out=wq_s[:, 0, 0:4], in_=wqv[:, 0, 0:4])
    nc.scalar.dma_start(out=wk_s[:, 0, 0:4], in_=wkv[:, 0, 0:4])
    nc.gpsimd.dma_start(out=xT_s[:, 0, 0], in_=xv[:, 0, 0])
    nc.gpsimd.dma_start(out=xT_s[:, 0, 1], in_=xv[:, 0, 1])
    nc.sync.dma_start(out=wq_s[:, 0, 4:8], in_=wqv[:, 0, 4:8])
    nc.scalar.dma_start(out=wk_s[:, 0, 4:8], in_=wkv[:, 0, 4:8])
    nc.sync.dma_start(out=xT_s[:, 0, 2], in_=xv[:, 0, 2])
    nc.scalar.dma_start(out=xT_s[:, 0, 3], in_=xv[:, 0, 3])
    nc.sync.dma_start(out=xT_s[:, 0, 4], in_=xv[:, 0, 4])
    nc.scalar.dma_start(out=xT_s[:, 0, 5], in_=xv[:, 0, 5])
    nc.sync.dma_start(out=xT_s[:, 0, 6], in_=xv[:, 0, 6])
    nc.scalar.dma_start(out=xT_s[:, 0, 7], in_=xv[:, 0, 7])
    nc.sync.dma_start(out=wv_s[:, 0:4], in_=wvv[:, 0:4])
    nc.scalar.dma_start(out=wv_s[:, 4:8], in_=wvv[:, 4:8])
    nc.sync.dma_start(out=wq_s[:, 1], in_=wqv[:, 1])
    nc.scalar.dma_start(out=wk_s[:, 1], in_=wkv[:, 1])
    for s4 in range(1, NQ):
        nc.sync.dma_start(out=xT_s[:, s4, 0:2], in_=xv[:, s4, 0:2])
        nc.scalar.dma_start(out=xT_s[:, s4, 4:6], in_=xv[:, s4, 4:6])
        nc.sync.dma_start(out=xT_s[:, s4, 2:4], in_=xv[:, s4, 2:4])
        nc.scalar.dma_start(out=xT_s[:, s4, 6:8], in_=xv[:, s4, 6:8])
    nc.sync.dma_start(out=wo_s, in_=wo_d.rearrange("p (c s) -> p c s", c=2))

    # ---- PE warm-up: junk matmuls engage the HAM clock ramp ASAP while
    # the first input pieces stream in; results are never read ----
    wp = qk_ps.tile([128, 512], F32, tag="qk", name="wp")
    for _ in range(10):
        nc.tensor.matmul(wp, lhsT=wu[:, 0:128], rhs=wu, start=True, stop=True)
    # ones columns of V: ctx matmul rows 0..63 = replicated denominator
    nc.gpsimd.memset(v_s[:, :, :, 0:HD], 1.0)

    def qk_one(pair, s4, which, pool=None):
        w_s, dst = ((wq_s, qt_s), (wk_s, kt_s))[which]
        pool = pool or qk_ps
        ps = pool.tile([128, 512], F32, tag=pool.name[:2], name="psqk")
        for ki in range(KI):
            nc.tensor.matmul(
                ps,
                lhsT=w_s[:, pair, ki, :],
                rhs=xT_s[:, s4, ki, :],
                start=(ki == 0),
                stop=(ki == KI - 1),
            )
        nc.vector.tensor_copy(
            out=dst[:, pair, 512 * s4 : 512 * (s4 + 1)], in_=ps
        )

    def v_proj(sb):
        s4, c = sb // 4, sb % 4
        ps = qk_ps.tile([128, 256], F32, tag="qk", name="psv")
        for ki in range(KI):
            nc.tensor.matmul(
                ps,
                lhsT=xT_s[:, s4, ki, 128 * c : 128 * (c + 1)],
                rhs=wv_s[:, ki, :],
                start=(ki == 0),
                stop=(ki == KI - 1),
            )
        nc.vector.tensor_copy(
            out=v_s[:, sb, :, HD:],
            in_=ps.rearrange("p (h d) -> p h d", h=H_LOC),
        )

    # Q0/K0 (pair 0, seq chunk 0): contraction chains interleaved per
    # ki in DMA piece-arrival order so both chase the incoming stream.
    psq = sc_ps.tile([128, 512], F32, tag="sc", name="psq0")
    psk = sc_ps.tile([128, 512], F32, tag="sc", name="psk0")
    ki_order = [0, 1, 2, 3, 4, 5, 6, 7]
    for idx, ki in enumerate(ki_order):
        for w_s, ps in ((wq_s, psq), (wk_s, psk)):
            nc.tensor.matmul(
                ps,
                lhsT=w_s[:, 0, ki, :],
                rhs=xT_s[:, 0, ki, :],
                start=(idx == 0),
                stop=(idx == KI - 1),
            )
    nc.vector.tensor_copy(out=kt_s[:, 0, 0:512], in_=psk)
    nc.vector.tensor_copy(out=qt_s[:, 0, 0:512], in_=psq)

    def outproj(m, s4, copy_eng="dve", pool=None, dma_eng=None):
        """Full out-proj tile (both pair chunks) -> bf16 -> DRAM."""
        pool = pool or qk_ps
        op = pool.tile([128, 512], F32, tag=pool.name[:2], name="psop")
        for c in range(2):
            nc.tensor.matmul(
                op,
                lhsT=wo_s[:, c, 128 * m : 128 * (m + 1)],
                rhs=ctxT_s[:, c, 512 * s4 : 512 * (s4 + 1)],
                start=(c == 0),
                stop=(c == 1),
            )
        ob = ob_pool.tile([128, 512], BF16, tag="ob")
        if copy_eng == "act":
            nc.scalar.copy(out=ob, in_=op)
        else:
            nc.vector.tensor_copy(out=ob, in_=op)
        (dma_eng or nc.sync).dma_start(
            out=outT_d[128 * m : 128 * (m + 1), 512 * s4 : 512 * (s4 + 1)],
            in_=ob,
        )

    def do_filler(f):
        if f[0] == "v":
            v_proj(f[1])
        elif f[0] == "qk0":
            qk_one(0, f[1], f[2])
        elif f[0] == "qk1":
            qk_one(1, f[1], f[2])
        else:
            outproj(f[1], f[2])

    # ---- attention: the two head-pairs' seq chunks are interleaved into
    # one global pipeline so the small, latency-bound chunks (jj=0/1) hide
    # inside the other pair's dense chunks and the scalar-engine exp load
    # stays smooth; ctx matmuls lag two k-blocks behind exp so the
    # in-order PE stream never blocks on the softmax ----
    CHUNKS = [(0, 0), (0, 1), (1, 0), (0, 2), (1, 1), (1, 2), (0, 3), (1, 3)]
    corder = {c: i for i, c in enumerate(CHUNKS)}
    cps = {}  # (pair, jj) -> pair-view of ctx PSUM tile
    pending = []  # (pair, jj, kb, ptv, off) ctx units not yet emitted
    done_norms = set()  # (pair, jj)

    def ctx_mms(pair, jj, kb, ptv, off):
        cpv = cps[(pair, jj)]
        for i in range(2):
            h = 2 * pair + i
            nc.tensor.matmul(
                cpv[:, i, off:512],
                lhsT=v_s[:, kb, h, :],
                rhs=ptv[:, i, off:512],
                start=(kb == 0),
                stop=(kb == 4 * (jj + 1) - 1),
            )  # rows 0-63: D replicated; rows 64-127: ctx

    def normalize(pair, jj):
        done_norms.add((pair, jj))
        cpv = cps.pop((pair, jj))
        cp2 = cpv.rearrange("p h q -> p (h q)")
        q0 = 512 * jj
        r64 = r64_pool.tile([64, 1024], F32, tag="r64")
        nc.vector.reciprocal_approx_fast(out=r64, in_=cp2[0:HD, :])
        for i in range(2):
            nc.vector.tensor_mul(
                out=ctxT_s[64 * i : 64 * i + 64, pair, q0 : q0 + 512],
                in0=cpv[HD:, i, :],
                in1=r64[:, 512 * i : 512 * (i + 1)],
            )

    units = []  # (pair, jj, kb)
    for pair, jj in CHUNKS:
        units += [(pair, jj, kb) for kb in range(4 * (jj + 1))]
    # fillers in DMA-arrival / deadline order; op tiles (both-pair
    # out-proj) go last and are gated on their chunk's normalizes
    fillers = [("v", 0), ("v", 1), ("v", 2), ("v", 3),
               ("qk1", 0, 0), ("qk1", 0, 1)]
    for s4 in range(1, NQ):
        fillers += [("qk0", s4, 0), ("qk0", s4, 1),
                    ("v", 4 * s4), ("v", 4 * s4 + 1),
                    ("v", 4 * s4 + 2), ("v", 4 * s4 + 3),
                    ("qk1", s4, 0), ("qk1", s4, 1)]
    for s4 in range(NQ - 1):
        for m in range(8):
            if s4 == NQ - 2 and m >= 4:
                continue  # held back as tail fillers
            fillers.append(("op", m, s4))

    nfill = 0
    fpos = {f: i for i, f in enumerate(fillers)}

    def flush_until(target):
        """Emit fillers up to and including target (no-op if emitted)."""
        nonlocal nfill
        while nfill <= fpos[target]:
            f = fillers[nfill]
            nfill += 1
            do_filler(f)

    def chunk_qk(c):
        pair, jj = c
        if (pair, jj) == (0, 0):
            return None  # covered by the psq/psk prologue
        return ("qk0", jj, 1) if pair == 0 else ("qk1", jj, 1)

    last_of_chunk = {}
    pos = 0
    for c in CHUNKS:
        pos += 4 * (c[1] + 1)
        last_of_chunk[pos - 1] = c
    next_chunk = {CHUNKS[i]: CHUNKS[i + 1] for i in range(len(CHUNKS) - 1)}

    for u, (pair, jj, kb) in enumerate(units):
        # preflush the next chunk's Q/K 4-5 units before this chunk ends
        # so the PSUM->SBUF copy latency hides behind remaining units and
        # the next chunk's first scores pair is ready the moment the PE
        # reaches it; only on odd units so it never splits a pair burst
        if u % 2 == 1:
            for k in (2, 3):
                if (u + k) in last_of_chunk:
                    nc_ = next_chunk.get(last_of_chunk[u + k])
                    if nc_ is not None and chunk_qk(nc_) is not None:
                        flush_until(chunk_qk(nc_))
                    break
        if kb == 0:
            tgt = chunk_qk((pair, jj))
            if tgt is not None:  # Q/K for this chunk must exist
                flush_until(tgt)
            cp = ctx_ps.tile([128, 1024], F32, tag="ctx", name="cp")
            cps[(pair, jj)] = _pair_view(cp)
        q0 = 512 * jj
        d = kb - 4 * jj
        off = max(0, 128 * d)
        sp = sc_ps.tile([128, 1024], F32, tag="sc", name="sp")
        spv = _pair_view(sp)
        # paired scores matmuls (row groups 0-1 / 2-3 concurrent)
        for i in range(2):
            nc.tensor.matmul(
                spv[:, i, off:512],
                lhsT=kt_s[64 * i : 64 * i + 64, pair, 128 * kb : 128 * (kb + 1)],
                rhs=qt_s[64 * i : 64 * i + 64, pair, q0 + off : q0 + 512],
                start=True,
                stop=True,
            )
        pt = pt_pool.tile([128, 1024], BF16, tag="pt")
        ptv = _pair_view(pt)
        nc.scalar.activation(
            out=ptv[:, :, off:512],
            in_=spv[:, :, off:512],
            func=EXP,
            scale=float(SCALE),
        )
        if d >= 0:  # diagonal block: zero k>q entries (Pool engine)
            nc.gpsimd.affine_select(
                out=ptv[:, :, off : off + 128],
                in_=ptv[:, :, off : off + 128],
                compare_op=mybir.AluOpType.is_ge,
                fill=0.0,
                base=0,
                pattern=[[0, 2], [1, 128]],
                channel_multiplier=-1,
            )
        pending.append((pair, jj, kb, ptv, off))
        # Fillers + ctx retires run only after ODD units so consecutive
        # scores pairs issue back-to-back on the PE (a pair exiting into a
        # full-row matmul pays a ~110ns LDWEIGHTS stall; pair->pair does
        # not, so bursting 2 pairs halves that penalty). sc_ps bufs=2
        # holds both bursts' score tiles; pt ring covers pending<=6.
        if u % 2 == 1:
            # drip-feed filler work; out-proj fillers wait until their seq
            # chunk has been normalized for BOTH pairs
            budget = 2 if u >= 56 else 1
            for _ in range(budget):
                if nfill >= len(fillers):
                    break
                f = fillers[nfill]
                if f[0] == "op" and not (
                    (0, f[2]) in done_norms and (1, f[2]) in done_norms
                ):
                    break
                nfill += 1
                do_filler(f)
            # retire ctx lag; finish a chunk fully once its last exp is in
            while len(pending) > 5 or (
                pending and corder[pending[0][:2]] < corder[(pair, jj)]
            ):
                pp, pj, pk, pptv, poff = pending.pop(0)
                flush_until(("v", pk))
                ctx_mms(pp, pj, pk, pptv, poff)
                if pk == 4 * (pj + 1) - 1:
                    normalize(pp, pj)
    for pp, pj, pk, pptv, poff in pending:
        flush_until(("v", pk))
        ctx_mms(pp, pj, pk, pptv, poff)
        if pk == 4 * (pj + 1) - 1:
            normalize(pp, pj)
    while nfill < len(fillers):  # drain any fillers that didn't fit
        f = fillers[nfill]
        nfill += 1
        do_filler(f)

    # ---- tail: out-proj of the last seq chunk. The pair-0 halves of the
    # first four tiles are emitted eagerly (they do not depend on the final
    # normalize), filling PE while DVE finishes the last softmax. ----
    s4 = NQ - 1
    # held-back s4=2 tiles bridge the PE over the final normalize latency
    # (they must precede the eager tiles, which pin all 4 sc/qk PSUM bufs;
    # copies on scalar/gpsimd so vector is clear for the final recip+muls;
    # DMA issues fan out over all four queues)
    outproj(4, NQ - 2, copy_eng="act", pool=sc_ps, dma_eng=nc.sync)
    outproj(6, NQ - 2, copy_eng="act", pool=qk_ps, dma_eng=nc.gpsimd)
    outproj(5, NQ - 2, copy_eng="act", pool=sc_ps, dma_eng=nc.scalar)
    outproj(7, NQ - 2, copy_eng="act", pool=qk_ps, dma_eng=nc.gpsimd)
    eager = []
    for m in range(4):
        pool = sc_ps if m % 2 else qk_ps
        op = pool.tile([128, 512], F32, tag=pool.name[:2], name="psop")
        nc.tensor.matmul(
            op,
            lhsT=wo_s[:, 0, 128 * m : 128 * (m + 1)],
            rhs=ctxT_s[:, 0, 512 * s4 : 512 * (s4 + 1)],
            start=True,
            stop=False,
        )
        eager.append(op)
    # copies and DMA issues round-robin over engines so the tail chain
    # (mm -> copy -> issue -> transfer) never serializes on one engine
    cp_engs = ["dve", "act", "dve", "act", "dve", "act", "dve", "act"]
    dma_engs = [nc.sync, nc.scalar, nc.gpsimd, nc.sync,
                nc.scalar, nc.gpsimd, nc.sync, nc.scalar]
    for m in range(4):
        op = eager[m]
        nc.tensor.matmul(
            op,
            lhsT=wo_s[:, 1, 128 * m : 128 * (m + 1)],
            rhs=ctxT_s[:, 1, 512 * s4 : 512 * (s4 + 1)],
            start=False,
            stop=True,
        )
        ob = ob_pool.tile([128, 512], BF16, tag="ob")
        ce = cp_engs[m]
        if ce == "act":
            nc.scalar.copy(out=ob, in_=op)
        else:
            nc.vector.tensor_copy(out=ob, in_=op)
        dma_engs[m].dma_start(
            out=outT_d[128 * m : 128 * (m + 1), 512 * s4 : 512 * (s4 + 1)],
            in_=ob,
        )
    for m in range(4, 8):
        pool = sc_ps if m % 2 else qk_ps
        op = pool.tile([128, 512], F32, tag=pool.name[:2], name="psop")
        for c in range(2):
            nc.tensor.matmul(
                op,
                lhsT=wo_s[:, c, 128 * m : 128 * (m + 1)],
                rhs=ctxT_s[:, c, 512 * s4 : 512 * (s4 + 1)],
                start=(c == 0),
                stop=(c == 1),
            )
        ob = ob_pool.tile([128, 512], BF16, tag="ob")
        if m < 6:
            ce = cp_engs[m]
            if ce == "act":
                nc.scalar.copy(out=ob, in_=op)
            else:
                nc.vector.tensor_copy(out=ob, in_=op)
            dma_engs[m].dma_start(
                out=outT_d[128 * m : 128 * (m + 1), 512 * s4 : 512 * (s4 + 1)],
                in_=ob,
            )
        else:
            # last two tiles: halve the copy across vector+scalar and the
            # DMA across two queues so the final drain chain is ~half
            nc.vector.tensor_copy(out=ob[:, 0:256], in_=op[:, 0:256])
            nc.scalar.copy(out=ob[:, 256:512], in_=op[:, 256:512])
            e0, e1 = (nc.sync, nc.scalar) if m == 6 else (nc.gpsimd, nc.sync)
            e0.dma_start(
                out=outT_d[128 * m : 128 * (m + 1), 512 * s4 : 512 * s4 + 256],
                in_=ob[:, 0:256],
            )
            e1.dma_start(
                out=outT_d[
                    128 * m : 128 * (m + 1), 512 * s4 + 256 : 512 * (s4 + 1)
                ],
                in_=ob[:, 256:512],
            )

    ctx.close()


_CACHED_NC = None


def _get_nc():
    global _CACHED_NC
    if _CACHED_NC is not None:
        return _CACHED_NC
    nc = bacc.Bacc(
        "TRN2", target_bir_lowering=False, debug=False, num_devices=N_CORES
    )
    xT_d = nc.dram_tensor("xT", [128, NQ * KI * 512], BF16, kind="ExternalInput").ap()
    wq_d = nc.dram_tensor("wq", [128, 2 * KI * 128], BF16, kind="ExternalInput").ap()
    wk_d = nc.dram_tensor("wk", [128, 2 * KI * 128], BF16, kind="ExternalInput").ap()
    wv_d = nc.dram_tensor("wv", [128, KI * DH], BF16, kind="ExternalInput").ap()
    wo_d = nc.dram_tensor("wo", [128, 2 * D_OUT], BF16, kind="ExternalInput").ap()
    outT_d = nc.dram_tensor("outT", [D_OUT, S], BF16, kind="ExternalOutput").ap()
    with tile.TileContext(nc) as tc:
        _build_body(nc, tc, xT_d, wq_d, wk_d, wv_d, wo_d, outT_d)
    nc.compile()
    _CACHED_NC = nc
    return nc


def _x_layout(a):
    """x[b].T [1024, 2048] -> [128, NQ*KI*512] seq-chunk-major pieces."""
    return np.ascontiguousarray(
        a.reshape(KI, 128, NQ, 512).transpose(1, 2, 0, 3).reshape(128, -1)
    )


def _w_pairs(a):
    """W shard [1024, 256] -> [128, 2*KI*128] pair-major chunks."""
    return np.ascontiguousarray(
        a.reshape(KI, 128, 2, 128).transpose(1, 2, 0, 3).reshape(128, -1)
    )


def _chunked(a):
    """[C*128, N] -> [128, C*N] (partition-major chunks, on-chip layout)."""
    c = a.shape[0] // 128
    return np.ascontiguousarray(
        a.reshape(c, 128, a.shape[1]).transpose(1, 0, 2).reshape(128, -1)
    )


def _make_in_maps(x, W_q, W_k, W_v, W_o):
    bf = ml_dtypes.bfloat16
    in_maps = []
    xT = [_x_layout(np.ascontiguousarray(x[b].T)).astype(bf) for b in range(B)]
    for c in range(N_CORES):
        b, g = c // 4, c % 4
        sl = slice(DH * g, DH * (g + 1))
        in_maps.append(
            {
                "xT": xT[b],
                "wq": _w_pairs(np.ascontiguousarray(W_q[:, sl])).astype(bf),
                "wk": _w_pairs(np.ascontiguousarray(W_k[:, sl])).astype(bf),
                "wv": _chunked(np.ascontiguousarray(W_v[:, sl])).astype(bf),
                "wo": _chunked(np.ascontiguousarray(W_o[sl, :])).astype(bf),
            }
        )
    return in_maps


def run_cores(x, W_q, W_k, W_v, W_o, **spmd_kwargs):
    """Compile (cached), run on 8 cores, return raw results object."""
    nc = _get_nc()
    in_maps = _make_in_maps(x, W_q, W_k, W_v, W_o)
    return run_bass_kernel_spmd(
        nc, in_maps, core_ids=list(range(N_CORES)), **spmd_kwargs
    )


def gather(results, b_o):
    out = np.empty((B, S, D_OUT), np.float32)
    for b in range(B):
        acc = results[4 * b]["outT"].astype(np.float32).copy()
        for g in range(1, 4):
            acc += results[4 * b + g]["outT"]
        out[b] = acc.T + b_o.astype(np.float32)[None, :]
    return out


def kernel(x, W_q, W_k, W_v, W_o, b_o):
    x = np.asarray(x)
    res = run_cores(
        x, np.asarray(W_q), np.asarray(W_k), np.asarray(W_v), np.asarray(W_o)
    )
    return gather(res.results, np.asarray(b_o))



# revision 36
# speedup vs baseline: 1.0197x; 1.0197x over previous
"""Multi-head self-attention (causal) Trainium2 Bass kernel, 8-core SPMD.

Problem: B=2, S=2048, D_IN=1024, D_OUT=1024, H=16 heads (hd=64), causal
softmax, out-proj with bias.

Sharding: core c -> (batch b = c // 4, head-group g = c % 4). Each core
computes 4 heads of one batch: data-parallel over b, tensor-parallel over
heads (W_q/W_k/W_v column shards, W_o row shard). Host sums the 4 partial
out-proj results per batch and adds b_o.

On-core layout is fully transposed (feature-major):
  xT   [D_IN, S]                (host pre-transposes x[b])
  Qt,Kt[hd*4, S]  = W^T @ xT    (pair-major: 2 sbuf tiles of [128, S])
  V    [S, hd*4]  (normal orientation, 64 ones-columns PREPENDED per
                   head: the ctx matmul emits the softmax denominator
                   replicated across PSUM partitions 0-63, so the
                   reciprocal reads PSUM directly at partition offset 0)
  St   [k, q] scores transposed; Pt = exp(St/8) in bf16, causal-masked
  in-place by a Pool-engine affine_select (fill=0) on diagonal blocks
  ctxT [hd*4, S] normalized via reciprocal_approx_fast straight from
  PSUM (no staging copies)
  outT [D_OUT, S] partial = Wo_shard^T @ ctxT  (host transposes back)

All matmul operands bf16 (fp32 PSUM accumulate); softmax in fp32.
Scores for a head pair are packed: head0 on PE row-groups 0-1, head1 on
2-3 (concurrent matmuls).

Startup is DMA-software-pipelined over THREE issue queues (sync/
scalar/gpsimd): wq/wk pair-0 stream finest-first, xT seq chunk 0 as
per-ki 128KB pieces, a junk-matmul burst engages the HAM clock ramp
with no idle gap (an idle >~0.5us resets the ramp), and the Q0/K0
projection chains chase the incoming pieces so real attention starts
~12us in (vs waiting for the full 6MB input). The two head-pairs' seq
chunks are interleaved into ONE global pipeline (chunk order
p0j0,p0j1,p1j0,p0j2,p1j1,p1j2,p0j3,p1j3) so the small latency-bound
chunks hide inside the other pair's dense chunks and the scalar-engine
exp load stays smooth. Scores pairs are emitted in bursts of two
(fillers + ctx retires only after odd units): entering/exiting a
row-group-packed pair costs ~100ns of non-hidden LDWEIGHTS each, so
bursting halves that penalty; burst depth is capped at 2 by the two
scores PSUM tiles, whose recycling waits on the serial scalar exp
chain. ctx matmuls retire with lag behind exp (pt ring of 8) so
select/exp latency never blocks the in-order PE stream. QKV
projection / out-projection tiles are interleaved as deadline-ordered
fillers with positional forced flushes; four held-back out-proj tiles
bridge the PE over the final softmax normalize, and tail copies/DMA
issues fan out over scalar+vector engines and all three DMA queues.
"""

import numpy as np
import ml_dtypes

import concourse.bass as bass
import concourse.bacc as bacc
import concourse.tile as tile
import concourse.mybir as mybir
from concourse.bass_utils import run_bass_kernel_spmd

N_CORES = 8
B, S, D_IN, D_OUT, H = 2, 2048, 1024, 1024, 16
H_LOC = 4  # heads per core
HD = 64
DH = H_LOC * HD  # 256 = d_out shard per core
KI = D_IN // 128  # 8 contraction chunks
NQ = S // 512  # 4 seq chunks of 512
NB = S // 128  # 16 seq blocks of 128
SCALE = 1.0 / np.sqrt(np.float32(HD))  # 0.125

BF16 = mybir.dt.bfloat16
F32 = mybir.dt.float32
EXP = mybir.ActivationFunctionType.Exp


def _pair_view(ap2d):
    """[128, 1024] tile -> [128, 2, 512] (head-major) view."""
    return ap2d.rearrange("p (h q) -> p h q", h=2)


def _build_body(nc, tc, xT_d, wq_d, wk_d, wv_d, wo_d, outT_d):
    from contextlib import ExitStack

    ctx = ExitStack()
    const = ctx.enter_context(tc.tile_pool(name="const", bufs=1))
    # PSUM: sc [128,1024]x2 = 4 banks; ctx [128,1024]x1 = 2; qk [128,512]x2 = 2
    sc_ps = ctx.enter_context(tc.tile_pool(name="sc_ps", bufs=2, space="PSUM"))
    ctx_ps = ctx.enter_context(tc.tile_pool(name="ctx_ps", bufs=1, space="PSUM"))
    qk_ps = ctx.enter_context(tc.tile_pool(name="qk_ps", bufs=2, space="PSUM"))
    pt_pool = ctx.enter_context(tc.tile_pool(name="pt", bufs=8))
    ob_pool = ctx.enter_context(tc.tile_pool(name="ob", bufs=6))
    r64_pool = ctx.enter_context(tc.tile_pool(name="r64", bufs=2))

    # ---- resident SBUF tensors ----
    xT_s = const.tile([128, NQ, KI, 512], BF16)
    wq_s = const.tile([128, 2, KI, 128], BF16)
    wk_s = const.tile([128, 2, KI, 128], BF16)
    wv_s = const.tile([128, KI, DH], BF16)
    wo_s = const.tile([128, 2, D_OUT], BF16)
    qt_s = const.tile([128, 2, S], BF16)  # [64*sub + hd, pair, q]
    kt_s = const.tile([128, 2, S], BF16)
    v_s = const.tile([128, NB, H_LOC, 2 * HD], BF16)  # cols 0:HD = ones x64
    ctxT_s = const.tile([128, 2, S], BF16)
    wu = const.tile([128, 512], BF16)

    # ---- input DMAs: seq-major pieces so Q0/K0 can start early.
    # FOUR issue queues (sync/scalar/vector/gpsimd). Seq chunk 0 streams
    # as per-ki 128KB pieces split across the vector+gpsimd queues so the
    # Q0/K0 contraction chains chase arrival; weights pair-0 pieces go
    # finest-first on sync/scalar; wv right after so the first V blocks
    # unblock ctx of chunk 0 quickly; wo goes last. ----
    xv = xT_d.rearrange("p (s k c) -> p s k c", s=NQ, k=KI)
    wqv = wq_d.rearrange("p (r k c) -> p r k c", r=2, k=KI)
    wkv = wk_d.rearrange("p (r k c) -> p r k c", r=2, k=KI)
    wvv = wv_d.rearrange("p (c s) -> p c s", c=KI)
    # junk-matmul operand memset on the (otherwise idle) vector engine so
    # gpsimd's first x-piece DMA issues go out immediately
    nc.vector.memset(wu, 0.0)
    nc.sync.dma_start(out=wq_s[:, 0, 0:4], in_=wqv[:, 0, 0:4])
    nc.scalar.dma_start(out=wk_s[:, 0, 0:4], in_=wkv[:, 0, 0:4])
    nc.gpsimd.dma_start(out=xT_s[:, 0, 0], in_=xv[:, 0, 0])
    nc.gpsimd.dma_start(out=xT_s[:, 0, 1], in_=xv[:, 0, 1])
    nc.sync.dma_start(out=wq_s[:, 0, 4:8], in_=wqv[:, 0, 4:8])
    nc.scalar.dma_start(out=wk_s[:, 0, 4:8], in_=wkv[:, 0, 4:8])
    nc.sync.dma_start(out=xT_s[:, 0, 2], in_=xv[:, 0, 2])
    nc.scalar.dma_start(out=xT_s[:, 0, 3], in_=xv[:, 0, 3])
    nc.sync.dma_start(out=xT_s[:, 0, 4], in_=xv[:, 0, 4])
    nc.scalar.dma_start(out=xT_s[:, 0, 5], in_=xv[:, 0, 5])
    nc.sync.dma_start(out=xT_s[:, 0, 6], in_=xv[:, 0, 6])
    nc.scalar.dma_start(out=xT_s[:, 0, 7], in_=xv[:, 0, 7])
    nc.sync.dma_start(out=wv_s[:, 0:4], in_=wvv[:, 0:4])
    nc.scalar.dma_start(out=wv_s[:, 4:8], in_=wvv[:, 4:8])
    nc.sync.dma_start(out=wq_s[:, 1], in_=wqv[:, 1])
    nc.scalar.dma_start(out=wk_s[:, 1], in_=wkv[:, 1])
    for s4 in range(1, NQ):
        nc.sync.dma_start(out=xT_s[:, s4, 0:2], in_=xv[:, s4, 0:2])
        nc.scalar.dma_start(out=xT_s[:, s4, 4:6], in_=xv[:, s4, 4:6])
        nc.sync.dma_start(out=xT_s[:, s4, 2:4], in_=xv[:, s4, 2:4])
        nc.scalar.dma_start(out=xT_s[:, s4, 6:8], in_=xv[:, s4, 6:8])
    nc.sync.dma_start(out=wo_s, in_=wo_d.rearrange("p (c s) -> p c s", c=2))

    # ---- PE warm-up: junk matmuls engage the HAM clock ramp ASAP while
    # the first input pieces stream in; results are never read ----
    wp = qk_ps.tile([128, 512], F32, tag="qk", name="wp")
    for _ in range(9):
        nc.tensor.matmul(wp, lhsT=wu[:, 0:128], rhs=wu, start=True, stop=True)
    # ones columns of V: ctx matmul rows 0..63 = replicated denominator
    nc.gpsimd.memset(v_s[:, :, :, 0:HD], 1.0)

    def qk_one(pair, s4, which, pool=None):
        w_s, dst = ((wq_s, qt_s), (wk_s, kt_s))[which]
        pool = pool or qk_ps
        ps = pool.tile([128, 512], F32, tag=pool.name[:2], name="psqk")
        for ki in range(KI):
            nc.tensor.matmul(
                ps,
                lhsT=w_s[:, pair, ki, :],
                rhs=xT_s[:, s4, ki, :],
                start=(ki == 0),
                stop=(ki == KI - 1),
            )
        nc.vector.tensor_copy(
            out=dst[:, pair, 512 * s4 : 512 * (s4 + 1)], in_=ps
        )

    def v_proj(sb):
        s4, c = sb // 4, sb % 4
        ps = qk_ps.tile([128, 256], F32, tag="qk", name="psv")
        for ki in range(KI):
            nc.tensor.matmul(
                ps,
                lhsT=xT_s[:, s4, ki, 128 * c : 128 * (c + 1)],
                rhs=wv_s[:, ki, :],
                start=(ki == 0),
                stop=(ki == KI - 1),
            )
        nc.vector.tensor_copy(
            out=v_s[:, sb, :, HD:],
            in_=ps.rearrange("p (h d) -> p h d", h=H_LOC),
        )

    # Q0/K0 (pair 0, seq chunk 0): contraction chains interleaved per
    # ki in DMA piece-arrival order so both chase the incoming stream.
    psq = sc_ps.tile([128, 512], F32, tag="sc", name="psq0")
    psk = sc_ps.tile([128, 512], F32, tag="sc", name="psk0")
    ki_order = [0, 1, 2, 3, 4, 5, 6, 7]
    for idx, ki in enumerate(ki_order):
        for w_s, ps in ((wq_s, psq), (wk_s, psk)):
            nc.tensor.matmul(
                ps,
                lhsT=w_s[:, 0, ki, :],
                rhs=xT_s[:, 0, ki, :],
                start=(idx == 0),
                stop=(idx == KI - 1),
            )
    nc.vector.tensor_copy(out=kt_s[:, 0, 0:512], in_=psk)
    nc.vector.tensor_copy(out=qt_s[:, 0, 0:512], in_=psq)

    def outproj(m, s4, copy_eng="dve", pool=None, dma_eng=None):
        """Full out-proj tile (both pair chunks) -> bf16 -> DRAM."""
        pool = pool or qk_ps
        op = pool.tile([128, 512], F32, tag=pool.name[:2], name="psop")
        for c in range(2):
            nc.tensor.matmul(
                op,
                lhsT=wo_s[:, c, 128 * m : 128 * (m + 1)],
                rhs=ctxT_s[:, c, 512 * s4 : 512 * (s4 + 1)],
                start=(c == 0),
                stop=(c == 1),
            )
        ob = ob_pool.tile([128, 512], BF16, tag="ob")
        if copy_eng == "act":
            nc.scalar.copy(out=ob, in_=op)
        else:
            nc.vector.tensor_copy(out=ob, in_=op)
        (dma_eng or nc.sync).dma_start(
            out=outT_d[128 * m : 128 * (m + 1), 512 * s4 : 512 * (s4 + 1)],
            in_=ob,
        )

    def do_filler(f):
        if f[0] == "v":
            v_proj(f[1])
        elif f[0] == "qk0":
            qk_one(0, f[1], f[2])
        elif f[0] == "qk1":
            qk_one(1, f[1], f[2])
        else:
            outproj(f[1], f[2])

    # ---- attention: the two head-pairs' seq chunks are interleaved into
    # one global pipeline so the small, latency-bound chunks (jj=0/1) hide
    # inside the other pair's dense chunks and the scalar-engine exp load
    # stays smooth; ctx matmuls lag two k-blocks behind exp so the
    # in-order PE stream never blocks on the softmax ----
    CHUNKS = [(0, 0), (0, 1), (1, 0), (0, 2), (1, 1), (1, 2), (0, 3), (1, 3)]
    corder = {c: i for i, c in enumerate(CHUNKS)}
    cps = {}  # (pair, jj) -> pair-view of ctx PSUM tile
    pending = []  # (pair, jj, kb, ptv, off) ctx units not yet emitted
    done_norms = set()  # (pair, jj)

    def ctx_mms(pair, jj, kb, ptv, off):
        cpv = cps[(pair, jj)]
        for i in range(2):
            h = 2 * pair + i
            nc.tensor.matmul(
                cpv[:, i, off:512],
                lhsT=v_s[:, kb, h, :],
                rhs=ptv[:, i, off:512],
                start=(kb == 0),
                stop=(kb == 4 * (jj + 1) - 1),
            )  # rows 0-63: D replicated; rows 64-127: ctx

    def normalize(pair, jj):
        done_norms.add((pair, jj))
        cpv = cps.pop((pair, jj))
        cp2 = cpv.rearrange("p h q -> p (h q)")
        q0 = 512 * jj
        r64 = r64_pool.tile([64, 1024], F32, tag="r64")
        nc.vector.reciprocal_approx_fast(out=r64, in_=cp2[0:HD, :])
        for i in range(2):
            nc.vector.tensor_mul(
                out=ctxT_s[64 * i : 64 * i + 64, pair, q0 : q0 + 512],
                in0=cpv[HD:, i, :],
                in1=r64[:, 512 * i : 512 * (i + 1)],
            )

    units = []  # (pair, jj, kb)
    for pair, jj in CHUNKS:
        units += [(pair, jj, kb) for kb in range(4 * (jj + 1))]
    # fillers in DMA-arrival / deadline order; op tiles (both-pair
    # out-proj) go last and are gated on their chunk's normalizes
    fillers = [("v", 0), ("v", 1), ("v", 2), ("v", 3),
               ("qk1", 0, 0), ("qk1", 0, 1)]
    for s4 in range(1, NQ):
        fillers += [("qk0", s4, 0), ("qk0", s4, 1),
                    ("v", 4 * s4), ("v", 4 * s4 + 1),
                    ("v", 4 * s4 + 2), ("v", 4 * s4 + 3),
                    ("qk1", s4, 0), ("qk1", s4, 1)]
    for s4 in range(NQ - 1):
        for m in range(8):
            if s4 == NQ - 2 and m >= 4:
                continue  # held back as tail fillers
            fillers.append(("op", m, s4))

    nfill = 0
    fpos = {f: i for i, f in enumerate(fillers)}

    def flush_until(target):
        """Emit fillers up to and including target (no-op if emitted)."""
        nonlocal nfill
        while nfill <= fpos[target]:
            f = fillers[nfill]
            nfill += 1
            do_filler(f)

    def chunk_qk(c):
        pair, jj = c
        if (pair, jj) == (0, 0):
            return None  # covered by the psq/psk prologue
        return ("qk0", jj, 1) if pair == 0 else ("qk1", jj, 1)

    last_of_chunk = {}
    pos = 0
    for c in CHUNKS:
        pos += 4 * (c[1] + 1)
        last_of_chunk[pos - 1] = c
    next_chunk = {CHUNKS[i]: CHUNKS[i + 1] for i in range(len(CHUNKS) - 1)}

    for u, (pair, jj, kb) in enumerate(units):
        # preflush the next chunk's Q/K 4-5 units before this chunk ends
        # so the PSUM->SBUF copy latency hides behind remaining units and
        # the next chunk's first scores pair is ready the moment the PE
        # reaches it; only on odd units so it never splits a pair burst
        if u % 2 == 1:
            for k in (2, 3):
                if (u + k) in last_of_chunk:
                    nc_ = next_chunk.get(last_of_chunk[u + k])
                    if nc_ is not None and chunk_qk(nc_) is not None:
                        flush_until(chunk_qk(nc_))
                    break
        if kb == 0:
            tgt = chunk_qk((pair, jj))
            if tgt is not None:  # Q/K for this chunk must exist
                flush_until(tgt)
            cp = ctx_ps.tile([128, 1024], F32, tag="ctx", name="cp")
            cps[(pair, jj)] = _pair_view(cp)
        q0 = 512 * jj
        d = kb - 4 * jj
        off = max(0, 128 * d)
        sp = sc_ps.tile([128, 1024], F32, tag="sc", name="sp")
        spv = _pair_view(sp)
        # paired scores matmuls (row groups 0-1 / 2-3 concurrent)
        for i in range(2):
            nc.tensor.matmul(
                spv[:, i, off:512],
                lhsT=kt_s[64 * i : 64 * i + 64, pair, 128 * kb : 128 * (kb + 1)],
                rhs=qt_s[64 * i : 64 * i + 64, pair, q0 + off : q0 + 512],
                start=True,
                stop=True,
            )
        pt = pt_pool.tile([128, 1024], BF16, tag="pt")
        ptv = _pair_view(pt)
        nc.scalar.activation(
            out=ptv[:, :, off:512],
            in_=spv[:, :, off:512],
            func=EXP,
            scale=float(SCALE),
        )
        if d >= 0:  # diagonal block: zero k>q entries (Pool engine)
            nc.gpsimd.affine_select(
                out=ptv[:, :, off : off + 128],
                in_=ptv[:, :, off : off + 128],
                compare_op=mybir.AluOpType.is_ge,
                fill=0.0,
                base=0,
                pattern=[[0, 2], [1, 128]],
                channel_multiplier=-1,
            )
        pending.append((pair, jj, kb, ptv, off))
        # Fillers + ctx retires run only after ODD units so consecutive
        # scores pairs issue back-to-back on the PE (a pair exiting into a
        # full-row matmul pays a ~110ns LDWEIGHTS stall; pair->pair does
        # not, so bursting 2 pairs halves that penalty). sc_ps bufs=2
        # holds both bursts' score tiles; pt ring covers pending<=6.
        if u % 2 == 1:
            # drip-feed filler work; out-proj fillers wait until their seq
            # chunk has been normalized for BOTH pairs
            budget = 2 if u >= 56 else 1
            for _ in range(budget):
                if nfill >= len(fillers):
                    break
                f = fillers[nfill]
                if f[0] == "op" and not (
                    (0, f[2]) in done_norms and (1, f[2]) in done_norms
                ):
                    break
                nfill += 1
                do_filler(f)
            # retire ctx lag; finish a chunk fully once its last exp is in
            while len(pending) > 5 or (
                pending and corder[pending[0][:2]] < corder[(pair, jj)]
            ):
                pp, pj, pk, pptv, poff = pending.pop(0)
                flush_until(("v", pk))
                ctx_mms(pp, pj, pk, pptv, poff)
                if pk == 4 * (pj + 1) - 1:
                    normalize(pp, pj)
    for pp, pj, pk, pptv, poff in pending:
        flush_until(("v", pk))
        ctx_mms(pp, pj, pk, pptv, poff)
        if pk == 4 * (pj + 1) - 1:
            normalize(pp, pj)
    while nfill < len(fillers):  # drain any fillers that didn't fit
        f = fillers[nfill]
        nfill += 1
        do_filler(f)

    # ---- tail: out-proj of the last seq chunk. The pair-0 halves of the
    # first four tiles are emitted eagerly (they do not depend on the final
    # normalize), filling PE while DVE finishes the last softmax. ----
    s4 = NQ - 1
    # held-back s4=2 tiles bridge the PE over the final normalize latency
    # (they must precede the eager tiles, which pin all 4 sc/qk PSUM bufs;
    # copies on scalar/gpsimd so vector is clear for the final recip+muls;
    # DMA issues fan out over all four queues)
    outproj(4, NQ - 2, copy_eng="act", pool=sc_ps, dma_eng=nc.sync)
    outproj(6, NQ - 2, copy_eng="act", pool=qk_ps, dma_eng=nc.gpsimd)
    outproj(5, NQ - 2, copy_eng="act", pool=sc_ps, dma_eng=nc.scalar)
    outproj(7, NQ - 2, copy_eng="act", pool=qk_ps, dma_eng=nc.gpsimd)
    eager = []
    for m in range(4):
        pool = sc_ps if m % 2 else qk_ps
        op = pool.tile([128, 512], F32, tag=pool.name[:2], name="psop")
        nc.tensor.matmul(
            op,
            lhsT=wo_s[:, 0, 128 * m : 128 * (m + 1)],
            rhs=ctxT_s[:, 0, 512 * s4 : 512 * (s4 + 1)],
            start=True,
            stop=False,
        )
        eager.append(op)
    # copies and DMA issues round-robin over engines so the tail chain
    # (mm -> copy -> issue -> transfer) never serializes on one engine
    cp_engs = ["dve", "act", "dve", "act", "dve", "act", "dve", "act"]
    dma_engs = [nc.sync, nc.scalar, nc.gpsimd, nc.sync,
                nc.scalar, nc.gpsimd, nc.sync, nc.scalar]
    for m in range(4):
        op = eager[m]
        nc.tensor.matmul(
            op,
            lhsT=wo_s[:, 1, 128 * m : 128 * (m + 1)],
            rhs=ctxT_s[:, 1, 512 * s4 : 512 * (s4 + 1)],
            start=False,
            stop=True,
        )
        ob = ob_pool.tile([128, 512], BF16, tag="ob")
        ce = cp_engs[m]
        if ce == "act":
            nc.scalar.copy(out=ob, in_=op)
        else:
            nc.vector.tensor_copy(out=ob, in_=op)
        dma_engs[m].dma_start(
            out=outT_d[128 * m : 128 * (m + 1), 512 * s4 : 512 * (s4 + 1)],
            in_=ob,
        )
    for m in range(4, 8):
        pool = sc_ps if m % 2 else qk_ps
        op = pool.tile([128, 512], F32, tag=pool.name[:2], name="psop")
        for c in range(2):
            nc.tensor.matmul(
                op,
                lhsT=wo_s[:, c, 128 * m : 128 * (m + 1)],
                rhs=ctxT_s[:, c, 512 * s4 : 512 * (s4 + 1)],
                start=(c == 0),
                stop=(c == 1),
            )
        ob = ob_pool.tile([128, 512], BF16, tag="ob")
        ce = cp_engs[m]
        if ce == "act":
            nc.scalar.copy(out=ob, in_=op)
        else:
            nc.vector.tensor_copy(out=ob, in_=op)
        dma_engs[m].dma_start(
            out=outT_d[128 * m : 128 * (m + 1), 512 * s4 : 512 * (s4 + 1)],
            in_=ob,
        )

    ctx.close()


_CACHED_NC = None


def _get_nc():
    global _CACHED_NC
    if _CACHED_NC is not None:
        return _CACHED_NC
    nc = bacc.Bacc(
        "TRN2", target_bir_lowering=False, debug=False, num_devices=N_CORES
    )
    xT_d = nc.dram_tensor("xT", [128, NQ * KI * 512], BF16, kind="ExternalInput").ap()
    wq_d = nc.dram_tensor("wq", [128, 2 * KI * 128], BF16, kind="ExternalInput").ap()
    wk_d = nc.dram_tensor("wk", [128, 2 * KI * 128], BF16, kind="ExternalInput").ap()
    wv_d = nc.dram_tensor("wv", [128, KI * DH], BF16, kind="ExternalInput").ap()
    wo_d = nc.dram_tensor("wo", [128, 2 * D_OUT], BF16, kind="ExternalInput").ap()
    outT_d = nc.dram_tensor("outT", [D_OUT, S], BF16, kind="ExternalOutput").ap()
    with tile.TileContext(nc) as tc:
        _build_body(nc, tc, xT_d, wq_d, wk_d, wv_d, wo_d, outT_d)
    nc.compile()
    _CACHED_NC = nc
    return nc


def _x_layout(a):
    """x[b].T [1024, 2048] -> [128, NQ*KI*512] seq-chunk-major pieces."""
    return np.ascontiguousarray(
        a.reshape(KI, 128, NQ, 512).transpose(1, 2, 0, 3).reshape(128, -1)
    )


def _w_pairs(a):
    """W shard [1024, 256] -> [128, 2*KI*128] pair-major chunks."""
    return np.ascontiguousarray(
        a.reshape(KI, 128, 2, 128).transpose(1, 2, 0, 3).reshape(128, -1)
    )


def _chunked(a):
    """[C*128, N] -> [128, C*N] (partition-major chunks, on-chip layout)."""
    c = a.shape[0] // 128
    return np.ascontiguousarray(
        a.reshape(c, 128, a.shape[1]).transpose(1, 0, 2).reshape(128, -1)
    )


def _make_in_maps(x, W_q, W_k, W_v, W_o):
    bf = ml_dtypes.bfloat16
    in_maps = []
    xT = [_x_layout(np.ascontiguousarray(x[b].T)).astype(bf) for b in range(B)]
    for c in range(N_CORES):
        b, g = c // 4, c % 4
        sl = slice(DH * g, DH * (g + 1))
        in_maps.append(
            {
                "xT": xT[b],
                "wq": _w_pairs(np.ascontiguousarray(W_q[:, sl])).astype(bf),
                "wk": _w_pairs(np.ascontiguousarray(W_k[:, sl])).astype(bf),
                "wv": _chunked(np.ascontiguousarray(W_v[:, sl])).astype(bf),
                "wo": _chunked(np.ascontiguousarray(W_o[sl, :])).astype(bf),
            }
        )
    return in_maps


def run_cores(x, W_q, W_k, W_v, W_o, **spmd_kwargs):
    """Compile (cached), run on 8 cores, return raw results object."""
    nc = _get_nc()
    in_maps = _make_in_maps(x, W_q, W_k, W_v, W_o)
    return run_bass_kernel_spmd(
        nc, in_maps, core_ids=list(range(N_CORES)), **spmd_kwargs
    )


def gather(results, b_o):
    out = np.empty((B, S, D_OUT), np.float32)
    for b in range(B):
        acc = results[4 * b]["outT"].astype(np.float32).copy()
        for g in range(1, 4):
            acc += results[4 * b + g]["outT"]
        out[b] = acc.T + b_o.astype(np.float32)[None, :]
    return out


def kernel(x, W_q, W_k, W_v, W_o, b_o):
    x = np.asarray(x)
    res = run_cores(
        x, np.asarray(W_q), np.asarray(W_k), np.asarray(W_v), np.asarray(W_o)
    )
    return gather(res.results, np.asarray(b_o))



# revision 37
# speedup vs baseline: 1.0269x; 1.0071x over previous
"""Multi-head self-attention (causal) Trainium2 Bass kernel, 8-core SPMD.

Problem: B=2, S=2048, D_IN=1024, D_OUT=1024, H=16 heads (hd=64), causal
softmax, out-proj with bias.

Sharding: core c -> (batch b = c // 4, head-group g = c % 4). Each core
computes 4 heads of one batch: data-parallel over b, tensor-parallel over
heads (W_q/W_k/W_v column shards, W_o row shard). Host sums the 4 partial
out-proj results per batch and adds b_o.

On-core layout is fully transposed (feature-major):
  xT   [D_IN, S]                (host pre-transposes x[b])
  Qt,Kt[hd*4, S]  = W^T @ xT    (pair-major: 2 sbuf tiles of [128, S])
  V    [S, hd*4]  (normal orientation, 64 ones-columns PREPENDED per
                   head: the ctx matmul emits the softmax denominator
                   replicated across PSUM partitions 0-63, so the
                   reciprocal reads PSUM directly at partition offset 0)
  St   [k, q] scores transposed; Pt = exp(St/8) in bf16, causal-masked
  in-place by a Pool-engine affine_select (fill=0) on diagonal blocks
  ctxT [hd*4, S] normalized via reciprocal_approx_fast straight from
  PSUM (no staging copies)
  outT [D_OUT, S] partial = Wo_shard^T @ ctxT  (host transposes back)

All matmul operands bf16 (fp32 PSUM accumulate); softmax in fp32.
Scores for a head pair are packed: head0 on PE row-groups 0-1, head1 on
2-3 (concurrent matmuls).

Startup is DMA-software-pipelined over THREE issue queues (sync/
scalar/gpsimd): wq/wk pair-0 stream finest-first, xT seq chunk 0 as
per-ki 128KB pieces, a junk-matmul burst engages the HAM clock ramp
with no idle gap (an idle >~0.5us resets the ramp), and the Q0/K0
projection chains chase the incoming pieces so real attention starts
~12us in (vs waiting for the full 6MB input). The two head-pairs' seq
chunks are interleaved into ONE global pipeline (chunk order
p0j0,p0j1,p1j0,p0j2,p1j1,p1j2,p0j3,p1j3) so the small latency-bound
chunks hide inside the other pair's dense chunks and the scalar-engine
exp load stays smooth. Scores pairs are emitted in bursts of two
(fillers + ctx retires only after odd units): entering/exiting a
row-group-packed pair costs ~100ns of non-hidden LDWEIGHTS each, so
bursting halves that penalty; burst depth is capped at 2 by the two
scores PSUM tiles, whose recycling waits on the serial scalar exp
chain. ctx matmuls retire with lag behind exp (pt ring of 8) so
select/exp latency never blocks the in-order PE stream. QKV
projection / out-projection tiles are interleaved as deadline-ordered
fillers with positional forced flushes; four held-back out-proj tiles
bridge the PE over the final softmax normalize, and tail copies/DMA
issues fan out over scalar+vector engines and all three DMA queues.
"""

import numpy as np
import ml_dtypes

import concourse.bass as bass
import concourse.bacc as bacc
import concourse.tile as tile
import concourse.mybir as mybir
from concourse.bass_utils import run_bass_kernel_spmd

N_CORES = 8
B, S, D_IN, D_OUT, H = 2, 2048, 1024, 1024, 16
H_LOC = 4  # heads per core
HD = 64
DH = H_LOC * HD  # 256 = d_out shard per core
KI = D_IN // 128  # 8 contraction chunks
NQ = S // 512  # 4 seq chunks of 512
NB = S // 128  # 16 seq blocks of 128
SCALE = 1.0 / np.sqrt(np.float32(HD))  # 0.125

BF16 = mybir.dt.bfloat16
F32 = mybir.dt.float32
EXP = mybir.ActivationFunctionType.Exp


def _pair_view(ap2d):
    """[128, 1024] tile -> [128, 2, 512] (head-major) view."""
    return ap2d.rearrange("p (h q) -> p h q", h=2)


def _build_body(nc, tc, xT_d, wq_d, wk_d, wv_d, wo_d, outT_d):
    from contextlib import ExitStack

    ctx = ExitStack()
    const = ctx.enter_context(tc.tile_pool(name="const", bufs=1))
    # PSUM: sc [128,1024]x2 = 4 banks; ctx [128,1024]x1 = 2; qk [128,512]x2 = 2
    sc_ps = ctx.enter_context(tc.tile_pool(name="sc_ps", bufs=2, space="PSUM"))
    ctx_ps = ctx.enter_context(tc.tile_pool(name="ctx_ps", bufs=1, space="PSUM"))
    qk_ps = ctx.enter_context(tc.tile_pool(name="qk_ps", bufs=2, space="PSUM"))
    pt_pool = ctx.enter_context(tc.tile_pool(name="pt", bufs=10))
    ob_pool = ctx.enter_context(tc.tile_pool(name="ob", bufs=6))
    r64_pool = ctx.enter_context(tc.tile_pool(name="r64", bufs=2))

    # ---- resident SBUF tensors ----
    xT_s = const.tile([128, NQ, KI, 512], BF16)
    wq_s = const.tile([128, 2, KI, 128], BF16)
    wk_s = const.tile([128, 2, KI, 128], BF16)
    wv_s = const.tile([128, KI, DH], BF16)
    wo_s = const.tile([128, 2, D_OUT], BF16)
    qt_s = const.tile([128, 2, S], BF16)  # [64*sub + hd, pair, q]
    kt_s = const.tile([128, 2, S], BF16)
    v_s = const.tile([128, NB, H_LOC, 2 * HD], BF16)  # cols 0:HD = ones x64
    ctxT_s = const.tile([128, 2, S], BF16)
    wu = const.tile([128, 512], BF16)

    # ---- input DMAs: seq-major pieces so Q0/K0 can start early.
    # FOUR issue queues (sync/scalar/vector/gpsimd). Seq chunk 0 streams
    # as per-ki 128KB pieces split across the vector+gpsimd queues so the
    # Q0/K0 contraction chains chase arrival; weights pair-0 pieces go
    # finest-first on sync/scalar; wv right after so the first V blocks
    # unblock ctx of chunk 0 quickly; wo goes last. ----
    xv = xT_d.rearrange("p (s k c) -> p s k c", s=NQ, k=KI)
    wqv = wq_d.rearrange("p (r k c) -> p r k c", r=2, k=KI)
    wkv = wk_d.rearrange("p (r k c) -> p r k c", r=2, k=KI)
    wvv = wv_d.rearrange("p (c s) -> p c s", c=KI)
    # junk-matmul operand memset on the (otherwise idle) vector engine so
    # gpsimd's first x-piece DMA issues go out immediately
    nc.vector.memset(wu, 0.0)
    nc.sync.dma_start(out=wq_s[:, 0, 0:4], in_=wqv[:, 0, 0:4])
    nc.scalar.dma_start(out=wk_s[:, 0, 0:4], in_=wkv[:, 0, 0:4])
    nc.gpsimd.dma_start(out=xT_s[:, 0, 0], in_=xv[:, 0, 0])
    nc.gpsimd.dma_start(out=xT_s[:, 0, 1], in_=xv[:, 0, 1])
    nc.sync.dma_start(out=wq_s[:, 0, 4:8], in_=wqv[:, 0, 4:8])
    nc.scalar.dma_start(out=wk_s[:, 0, 4:8], in_=wkv[:, 0, 4:8])
    nc.sync.dma_start(out=xT_s[:, 0, 2], in_=xv[:, 0, 2])
    nc.scalar.dma_start(out=xT_s[:, 0, 3], in_=xv[:, 0, 3])
    nc.sync.dma_start(out=xT_s[:, 0, 4], in_=xv[:, 0, 4])
    nc.scalar.dma_start(out=xT_s[:, 0, 5], in_=xv[:, 0, 5])
    nc.sync.dma_start(out=xT_s[:, 0, 6], in_=xv[:, 0, 6])
    nc.scalar.dma_start(out=xT_s[:, 0, 7], in_=xv[:, 0, 7])
    nc.sync.dma_start(out=wv_s[:, 0:4], in_=wvv[:, 0:4])
    nc.scalar.dma_start(out=wv_s[:, 4:8], in_=wvv[:, 4:8])
    nc.sync.dma_start(out=wq_s[:, 1], in_=wqv[:, 1])
    nc.scalar.dma_start(out=wk_s[:, 1], in_=wkv[:, 1])
    for s4 in range(1, NQ):
        nc.sync.dma_start(out=xT_s[:, s4, 0:2], in_=xv[:, s4, 0:2])
        nc.scalar.dma_start(out=xT_s[:, s4, 4:6], in_=xv[:, s4, 4:6])
        nc.sync.dma_start(out=xT_s[:, s4, 2:4], in_=xv[:, s4, 2:4])
        nc.scalar.dma_start(out=xT_s[:, s4, 6:8], in_=xv[:, s4, 6:8])
    nc.sync.dma_start(out=wo_s, in_=wo_d.rearrange("p (c s) -> p c s", c=2))

    # ---- PE warm-up: junk matmuls engage the HAM clock ramp ASAP while
    # the first input pieces stream in; results are never read ----
    wp = qk_ps.tile([128, 512], F32, tag="qk", name="wp")
    for _ in range(9):
        nc.tensor.matmul(wp, lhsT=wu[:, 0:128], rhs=wu, start=True, stop=True)
    # ones columns of V: ctx matmul rows 0..63 = replicated denominator
    nc.gpsimd.memset(v_s[:, :, :, 0:HD], 1.0)

    def qk_one(pair, s4, which, pool=None):
        w_s, dst = ((wq_s, qt_s), (wk_s, kt_s))[which]
        pool = pool or qk_ps
        ps = pool.tile([128, 512], F32, tag=pool.name[:2], name="psqk")
        for ki in range(KI):
            nc.tensor.matmul(
                ps,
                lhsT=w_s[:, pair, ki, :],
                rhs=xT_s[:, s4, ki, :],
                start=(ki == 0),
                stop=(ki == KI - 1),
            )
        nc.vector.tensor_copy(
            out=dst[:, pair, 512 * s4 : 512 * (s4 + 1)], in_=ps
        )

    def v_proj(sb):
        s4, c = sb // 4, sb % 4
        ps = qk_ps.tile([128, 256], F32, tag="qk", name="psv")
        for ki in range(KI):
            nc.tensor.matmul(
                ps,
                lhsT=xT_s[:, s4, ki, 128 * c : 128 * (c + 1)],
                rhs=wv_s[:, ki, :],
                start=(ki == 0),
                stop=(ki == KI - 1),
            )
        nc.vector.tensor_copy(
            out=v_s[:, sb, :, HD:],
            in_=ps.rearrange("p (h d) -> p h d", h=H_LOC),
        )

    # Q0/K0 (pair 0, seq chunk 0): contraction chains interleaved per
    # ki in DMA piece-arrival order so both chase the incoming stream.
    psq = sc_ps.tile([128, 512], F32, tag="sc", name="psq0")
    psk = sc_ps.tile([128, 512], F32, tag="sc", name="psk0")
    ki_order = [0, 1, 2, 3, 4, 5, 6, 7]
    for idx, ki in enumerate(ki_order):
        for w_s, ps in ((wq_s, psq), (wk_s, psk)):
            nc.tensor.matmul(
                ps,
                lhsT=w_s[:, 0, ki, :],
                rhs=xT_s[:, 0, ki, :],
                start=(idx == 0),
                stop=(idx == KI - 1),
            )
    nc.vector.tensor_copy(out=kt_s[:, 0, 0:512], in_=psk)
    nc.vector.tensor_copy(out=qt_s[:, 0, 0:512], in_=psq)

    def outproj(m, s4, copy_eng="dve", pool=None, dma_eng=None):
        """Full out-proj tile (both pair chunks) -> bf16 -> DRAM."""
        pool = pool or qk_ps
        op = pool.tile([128, 512], F32, tag=pool.name[:2], name="psop")
        for c in range(2):
            nc.tensor.matmul(
                op,
                lhsT=wo_s[:, c, 128 * m : 128 * (m + 1)],
                rhs=ctxT_s[:, c, 512 * s4 : 512 * (s4 + 1)],
                start=(c == 0),
                stop=(c == 1),
            )
        ob = ob_pool.tile([128, 512], BF16, tag="ob")
        if copy_eng == "act":
            nc.scalar.copy(out=ob, in_=op)
        else:
            nc.vector.tensor_copy(out=ob, in_=op)
        (dma_eng or nc.sync).dma_start(
            out=outT_d[128 * m : 128 * (m + 1), 512 * s4 : 512 * (s4 + 1)],
            in_=ob,
        )

    def do_filler(f):
        if f[0] == "v":
            v_proj(f[1])
        elif f[0] == "qk0":
            qk_one(0, f[1], f[2])
        elif f[0] == "qk1":
            qk_one(1, f[1], f[2])
        else:
            outproj(f[1], f[2])

    # ---- attention: the two head-pairs' seq chunks are interleaved into
    # one global pipeline so the small, latency-bound chunks (jj=0/1) hide
    # inside the other pair's dense chunks and the scalar-engine exp load
    # stays smooth; ctx matmuls lag two k-blocks behind exp so the
    # in-order PE stream never blocks on the softmax ----
    CHUNKS = [(0, 0), (0, 1), (1, 0), (0, 2), (1, 1), (1, 2), (0, 3), (1, 3)]
    corder = {c: i for i, c in enumerate(CHUNKS)}
    cps = {}  # (pair, jj) -> pair-view of ctx PSUM tile
    pending = []  # (pair, jj, kb, ptv, off) ctx units not yet emitted
    done_norms = set()  # (pair, jj)

    def ctx_mms(pair, jj, kb, ptv, off):
        cpv = cps[(pair, jj)]
        for i in range(2):
            h = 2 * pair + i
            nc.tensor.matmul(
                cpv[:, i, off:512],
                lhsT=v_s[:, kb, h, :],
                rhs=ptv[:, i, off:512],
                start=(kb == 0),
                stop=(kb == 4 * (jj + 1) - 1),
            )  # rows 0-63: D replicated; rows 64-127: ctx

    def normalize(pair, jj):
        done_norms.add((pair, jj))
        cpv = cps.pop((pair, jj))
        cp2 = cpv.rearrange("p h q -> p (h q)")
        q0 = 512 * jj
        r64 = r64_pool.tile([64, 1024], F32, tag="r64")
        nc.vector.reciprocal_approx_fast(out=r64, in_=cp2[0:HD, :])
        for i in range(2):
            nc.vector.tensor_mul(
                out=ctxT_s[64 * i : 64 * i + 64, pair, q0 : q0 + 512],
                in0=cpv[HD:, i, :],
                in1=r64[:, 512 * i : 512 * (i + 1)],
            )

    units = []  # (pair, jj, kb)
    for pair, jj in CHUNKS:
        units += [(pair, jj, kb) for kb in range(4 * (jj + 1))]
    # fillers in DMA-arrival / deadline order; op tiles (both-pair
    # out-proj) go last and are gated on their chunk's normalizes
    fillers = [("v", 0), ("v", 1), ("v", 2), ("v", 3),
               ("qk1", 0, 0), ("qk1", 0, 1)]
    for s4 in range(1, NQ):
        fillers += [("qk0", s4, 0), ("qk0", s4, 1),
                    ("v", 4 * s4), ("v", 4 * s4 + 1),
                    ("v", 4 * s4 + 2), ("v", 4 * s4 + 3),
                    ("qk1", s4, 0), ("qk1", s4, 1)]
    for s4 in range(NQ - 1):
        for m in range(8):
            if s4 == NQ - 2 and m >= 4:
                continue  # held back as tail fillers
            fillers.append(("op", m, s4))

    nfill = 0
    fpos = {f: i for i, f in enumerate(fillers)}

    def flush_until(target):
        """Emit fillers up to and including target (no-op if emitted)."""
        nonlocal nfill
        while nfill <= fpos[target]:
            f = fillers[nfill]
            nfill += 1
            do_filler(f)

    def chunk_qk(c):
        pair, jj = c
        if (pair, jj) == (0, 0):
            return None  # covered by the psq/psk prologue
        return ("qk0", jj, 1) if pair == 0 else ("qk1", jj, 1)

    last_of_chunk = {}
    pos = 0
    for c in CHUNKS:
        pos += 4 * (c[1] + 1)
        last_of_chunk[pos - 1] = c
    next_chunk = {CHUNKS[i]: CHUNKS[i + 1] for i in range(len(CHUNKS) - 1)}

    for u, (pair, jj, kb) in enumerate(units):
        # preflush the next chunk's Q/K 4-5 units before this chunk ends
        # so the PSUM->SBUF copy latency hides behind remaining units and
        # the next chunk's first scores pair is ready the moment the PE
        # reaches it; only on odd units so it never splits a pair burst
        if u % 2 == 1:
            for k in (2, 3):
                if (u + k) in last_of_chunk:
                    nc_ = next_chunk.get(last_of_chunk[u + k])
                    if nc_ is not None and chunk_qk(nc_) is not None:
                        flush_until(chunk_qk(nc_))
                    break
        if kb == 0:
            tgt = chunk_qk((pair, jj))
            if tgt is not None:  # Q/K for this chunk must exist
                flush_until(tgt)
            cp = ctx_ps.tile([128, 1024], F32, tag="ctx", name="cp")
            cps[(pair, jj)] = _pair_view(cp)
        q0 = 512 * jj
        d = kb - 4 * jj
        off = max(0, 128 * d)
        sp = sc_ps.tile([128, 1024], F32, tag="sc", name="sp")
        spv = _pair_view(sp)
        # paired scores matmuls (row groups 0-1 / 2-3 concurrent)
        for i in range(2):
            nc.tensor.matmul(
                spv[:, i, off:512],
                lhsT=kt_s[64 * i : 64 * i + 64, pair, 128 * kb : 128 * (kb + 1)],
                rhs=qt_s[64 * i : 64 * i + 64, pair, q0 + off : q0 + 512],
                start=True,
                stop=True,
            )
        pt = pt_pool.tile([128, 1024], BF16, tag="pt")
        ptv = _pair_view(pt)
        nc.scalar.activation(
            out=ptv[:, :, off:512],
            in_=spv[:, :, off:512],
            func=EXP,
            scale=float(SCALE),
        )
        if d >= 0:  # diagonal block: zero k>q entries (Pool engine)
            nc.gpsimd.affine_select(
                out=ptv[:, :, off : off + 128],
                in_=ptv[:, :, off : off + 128],
                compare_op=mybir.AluOpType.is_ge,
                fill=0.0,
                base=0,
                pattern=[[0, 2], [1, 128]],
                channel_multiplier=-1,
            )
        pending.append((pair, jj, kb, ptv, off))
        # Fillers + ctx retires run only after ODD units so consecutive
        # scores pairs issue back-to-back on the PE (a pair exiting into a
        # full-row matmul pays a ~110ns LDWEIGHTS stall; pair->pair does
        # not, so bursting 2 pairs halves that penalty). sc_ps bufs=2
        # holds both bursts' score tiles; pt ring covers pending<=6.
        if u % 2 == 1:
            # drip-feed filler work; out-proj fillers wait until their seq
            # chunk has been normalized for BOTH pairs
            budget = 2 if u >= 56 else 1
            for _ in range(budget):
                if nfill >= len(fillers):
                    break
                f = fillers[nfill]
                if f[0] == "op" and not (
                    (0, f[2]) in done_norms and (1, f[2]) in done_norms
                ):
                    break
                nfill += 1
                do_filler(f)
            # retire ctx lag; finish a chunk fully once its last exp is in
            while len(pending) > 5 or (
                pending and corder[pending[0][:2]] < corder[(pair, jj)]
            ):
                pp, pj, pk, pptv, poff = pending.pop(0)
                flush_until(("v", pk))
                ctx_mms(pp, pj, pk, pptv, poff)
                if pk == 4 * (pj + 1) - 1:
                    normalize(pp, pj)
    for pp, pj, pk, pptv, poff in pending:
        flush_until(("v", pk))
        ctx_mms(pp, pj, pk, pptv, poff)
        if pk == 4 * (pj + 1) - 1:
            normalize(pp, pj)
    while nfill < len(fillers):  # drain any fillers that didn't fit
        f = fillers[nfill]
        nfill += 1
        do_filler(f)

    # ---- tail: out-proj of the last seq chunk. The pair-0 halves of the
    # first four tiles are emitted eagerly (they do not depend on the final
    # normalize), filling PE while DVE finishes the last softmax. ----
    s4 = NQ - 1
    # held-back s4=2 tiles bridge the PE over the final normalize latency
    # (they must precede the eager tiles, which pin all 4 sc/qk PSUM bufs;
    # copies on scalar/gpsimd so vector is clear for the final recip+muls;
    # DMA issues fan out over all four queues)
    outproj(4, NQ - 2, copy_eng="act", pool=sc_ps, dma_eng=nc.sync)
    outproj(6, NQ - 2, copy_eng="act", pool=qk_ps, dma_eng=nc.gpsimd)
    outproj(5, NQ - 2, copy_eng="act", pool=sc_ps, dma_eng=nc.scalar)
    outproj(7, NQ - 2, copy_eng="act", pool=qk_ps, dma_eng=nc.gpsimd)
    eager = []
    for m in range(4):
        pool = sc_ps if m % 2 else qk_ps
        op = pool.tile([128, 512], F32, tag=pool.name[:2], name="psop")
        nc.tensor.matmul(
            op,
            lhsT=wo_s[:, 0, 128 * m : 128 * (m + 1)],
            rhs=ctxT_s[:, 0, 512 * s4 : 512 * (s4 + 1)],
            start=True,
            stop=False,
        )
        eager.append(op)
    # copies and DMA issues round-robin over engines so the tail chain
    # (mm -> copy -> issue -> transfer) never serializes on one engine
    cp_engs = ["dve", "act", "dve", "act", "dve", "act", "dve", "act"]
    dma_engs = [nc.sync, nc.scalar, nc.gpsimd, nc.sync,
                nc.scalar, nc.gpsimd, nc.sync, nc.scalar]
    for m in range(4):
        op = eager[m]
        nc.tensor.matmul(
            op,
            lhsT=wo_s[:, 1, 128 * m : 128 * (m + 1)],
            rhs=ctxT_s[:, 1, 512 * s4 : 512 * (s4 + 1)],
            start=False,
            stop=True,
        )
        ob = ob_pool.tile([128, 512], BF16, tag="ob")
        ce = cp_engs[m]
        if ce == "act":
            nc.scalar.copy(out=ob, in_=op)
        else:
            nc.vector.tensor_copy(out=ob, in_=op)
        dma_engs[m].dma_start(
            out=outT_d[128 * m : 128 * (m + 1), 512 * s4 : 512 * (s4 + 1)],
            in_=ob,
        )
    for m in range(4, 8):
        pool = sc_ps if m % 2 else qk_ps
        op = pool.tile([128, 512], F32, tag=pool.name[:2], name="psop")
        for c in range(2):
            nc.tensor.matmul(
                op,
                lhsT=wo_s[:, c, 128 * m : 128 * (m + 1)],
                rhs=ctxT_s[:, c, 512 * s4 : 512 * (s4 + 1)],
                start=(c == 0),
                stop=(c == 1),
            )
        ob = ob_pool.tile([128, 512], BF16, tag="ob")
        ce = cp_engs[m]
        if ce == "act":
            nc.scalar.copy(out=ob, in_=op)
        else:
            nc.vector.tensor_copy(out=ob, in_=op)
        dma_engs[m].dma_start(
            out=outT_d[128 * m : 128 * (m + 1), 512 * s4 : 512 * (s4 + 1)],
            in_=ob,
        )

    ctx.close()


_CACHED_NC = None


def _get_nc():
    global _CACHED_NC
    if _CACHED_NC is not None:
        return _CACHED_NC
    nc = bacc.Bacc(
        "TRN2", target_bir_lowering=False, debug=False, num_devices=N_CORES
    )
    xT_d = nc.dram_tensor("xT", [128, NQ * KI * 512], BF16, kind="ExternalInput").ap()
    wq_d = nc.dram_tensor("wq", [128, 2 * KI * 128], BF16, kind="ExternalInput").ap()
    wk_d = nc.dram_tensor("wk", [128, 2 * KI * 128], BF16, kind="ExternalInput").ap()
    wv_d = nc.dram_tensor("wv", [128, KI * DH], BF16, kind="ExternalInput").ap()
    wo_d = nc.dram_tensor("wo", [128, 2 * D_OUT], BF16, kind="ExternalInput").ap()
    outT_d = nc.dram_tensor("outT", [D_OUT, S], BF16, kind="ExternalOutput").ap()
    with tile.TileContext(nc) as tc:
        _build_body(nc, tc, xT_d, wq_d, wk_d, wv_d, wo_d, outT_d)
    nc.compile()
    _CACHED_NC = nc
    return nc


def _x_layout(a):
    """x[b].T [1024, 2048] -> [128, NQ*KI*512] seq-chunk-major pieces."""
    return np.ascontiguousarray(
        a.reshape(KI, 128, NQ, 512).transpose(1, 2, 0, 3).reshape(128, -1)
    )


def _w_pairs(a):
    """W shard [1024, 256] -> [128, 2*KI*128] pair-major chunks."""
    return np.ascontiguousarray(
        a.reshape(KI, 128, 2, 128).transpose(1, 2, 0, 3).reshape(128, -1)
    )


def _chunked(a):
    """[C*128, N] -> [128, C*N] (partition-major chunks, on-chip layout)."""
    c = a.shape[0] // 128
    return np.ascontiguousarray(
        a.reshape(c, 128, a.shape[1]).transpose(1, 0, 2).reshape(128, -1)
    )


def _make_in_maps(x, W_q, W_k, W_v, W_o):
    bf = ml_dtypes.bfloat16
    in_maps = []
    xT = [_x_layout(np.ascontiguousarray(x[b].T)).astype(bf) for b in range(B)]
    for c in range(N_CORES):
        b, g = c // 4, c % 4
        sl = slice(DH * g, DH * (g + 1))
        in_maps.append(
            {
                "xT": xT[b],
                "wq": _w_pairs(np.ascontiguousarray(W_q[:, sl])).astype(bf),
                "wk": _w_pairs(np.ascontiguousarray(W_k[:, sl])).astype(bf),
                "wv": _chunked(np.ascontiguousarray(W_v[:, sl])).astype(bf),
                "wo": _chunked(np.ascontiguousarray(W_o[sl, :])).astype(bf),
            }
        )
    return in_maps


def run_cores(x, W_q, W_k, W_v, W_o, **spmd_kwargs):
    """Compile (cached), run on 8 cores, return raw results object."""
    nc = _get_nc()
    in_maps = _make_in_maps(x, W_q, W_k, W_v, W_o)
    return run_bass_kernel_spmd(
        nc, in_maps, core_ids=list(range(N_CORES)), **spmd_kwargs
    )


def gather(results, b_o):
    out = np.empty((B, S, D_OUT), np.float32)
    for b in range(B):
        acc = results[4 * b]["outT"].astype(np.float32).copy()
        for g in range(1, 4):
            acc += results[4 * b + g]["outT"]
        out[b] = acc.T + b_o.astype(np.float32)[None, :]
    return out


def kernel(x, W_q, W_k, W_v, W_o, b_o):
    x = np.asarray(x)
    res = run_cores(
        x, np.asarray(W_q), np.asarray(W_k), np.asarray(W_v), np.asarray(W_o)
    )
    return gather(res.results, np.asarray(b_o))

